# revision 39
# baseline (speedup 1.0000x reference)
"""Trainium2 Bass kernel for a quantized ResNet BasicBlock.

Reference computation (per reference.py):
    out = act_quant(x); out = conv3x3(out, weight_quant(w1)); out = BN(out, g1, b1)
    out = act_quant(out); out = conv3x3(out, weight_quant(w2)); out = BN(out, g2, b2)
    return out + x
with act_quant(x) = round(clip(x,0,1)*15)/15 (4-bit), weight_quant symmetric 4-bit
per-tensor (levels -7..7, scale alpha/7, alpha = max|w|), BN in training mode
(batch stats over (N,H,W)).

Strategy (8 NeuronCores, data-parallel over batch, sync-BN via AllReduce):
  * Quantized activations are integers 0..15, weights integers -7..7 - both
    exact in fp8e4m3, and fp32 PSUM accumulation never rounds, so each conv3x3
    is an EXACT integer computation.
  * act_quant in 2 ops: float->uint8 conversion saturates [0,255] and rounds
    RNE (matches jnp.round + bottom clip), then min(u8,15) -> fp8 gives the
    top clip.  No +128 bias trick, no separate clip pass.
  * conv3x3 over a zero-padded [C=128, 58, 64] fp8 image as 5 dense DoubleRow
    pair-matmuls per 8-row group (out [C,8,56], 448 wide): pairs (0,dw)+(1,dw)
    at pair-stride 64, (2,0)+(2,2) at pair-stride 2, and (2,1)+zero-weight-row
    at stride 2.  All 9 taps run at the fp8 DoubleRow rate.
  * PSUM->SBUF copy (Act, accum_out) emits per-channel sums and stores conv
    results as int16; sum-of-squares via DVE scalar_tensor_tensor / Act Square
    (split for engine balance, emitted two images behind the conv so in-order
    engine queues never convoy quant ops behind the heavy stats ops).
    Per-channel sum/sumsq are AllReduced across the 8 cores ([128,2] fp32);
    BN+act_quant collapse into a per-channel scale/bias.
  * Finalize: out = fscale*cint + (x + fbias) via Act bias-add + DVE
    scalar_tensor_tensor (6 images) or DVE affine + Pool add (2 images),
    written as bf16 (host upcasts; ~2e-3 rel err, gate is 2e-2) which halves
    the store DMA.  x stays resident in SBUF the whole time (no reload).
  * Dummy DoubleRow matmuls on the constant weight tile keep the tensor
    engine's p-state ramp alive at program start and across the BN1 barrier.
"""

import os
import sys

for _p in ("/opt/trn_rl_repo", "/root/.axon_site/_ro/trn_rl_repo"):
    if os.path.isdir(_p) and _p not in sys.path:
        sys.path.insert(0, _p)

import numpy as np
import ml_dtypes

import concourse.bass as bass  # noqa: F401  (registers types)
import concourse.tile as tile
from concourse import bacc, mybir
from concourse import bass_utils

F32 = mybir.dt.float32
BF16 = mybir.dt.bfloat16
I16 = mybir.dt.int16
U8 = mybir.dt.uint8
F8 = mybir.dt.float8e4
ACTF = mybir.ActivationFunctionType
ALU = mybir.AluOpType
AX = mybir.AxisListType
DR = mybir.MatmulPerfMode.DoubleRow

C = 128
H = W = 56
HP = 58               # padded rows: 1 top + 56 + 1 bottom
WP = 64               # padded cols (16B-aligned rows)
GR = 8                # output rows per PSUM group
NG = H // GR          # 7 groups per image
NCORES = 8

# cvec column indices (all [C] fp32, host-computed)
CV_G1, CV_B15, CV_G2, CV_B2, CV_S1SQ, CV_S2SQ, CV_S15, CV_S2, CV_NCOLS = range(9)

BN_EPS = 1e-5

# DoubleRow tap pairs: (flat offset within group, pair stride).  Weight rows
# 2p,2p+1 hold the two taps; row 9 is all-zero (pairs tap (2,1) with garbage).
PAIRS = [(0 * WP + 0, WP),   # (0,0)+(1,0)
         (0 * WP + 1, WP),   # (0,1)+(1,1)
         (0 * WP + 2, WP),   # (0,2)+(1,2)
         (2 * WP + 0, 2),    # (2,0)+(2,2)
         (2 * WP + 1, 2)]    # (2,1)+zero


def _bn_coefs(nc, pool, S, SS, cvcol, inv_m, ph):
    """[C,1] coef math from global integer-unit sum S / sumsq SS.

    ph=1: (uscale, ubias) with u = conv_int*uscale + ubias = 15*BN(y); u8
          conversion then rounds and bottom-clips, min(,15) top-clips.
    ph=2: (fscale, fbias) with out = conv_int*fscale + fbias = BN(y2).
    """
    idx = [0]

    def mk():
        idx[0] += 1
        return pool.tile([C, 1], F32, tag=f"bc{ph}_{idx[0]}", name=f"bc{ph}_{idx[0]}")

    mean = mk()
    nc.vector.tensor_scalar(mean[:], S, inv_m, None, op0=ALU.mult)
    e2 = mk()
    nc.vector.tensor_scalar(e2[:], SS, inv_m, None, op0=ALU.mult)
    msq = mk()
    nc.vector.tensor_tensor(out=msq[:], in0=mean[:], in1=mean[:], op=ALU.mult)
    var = mk()
    nc.vector.tensor_tensor(out=var[:], in0=e2[:], in1=msq[:], op=ALU.subtract)
    v = mk()
    nc.vector.tensor_scalar(v[:], var[:], cvcol(CV_S1SQ if ph == 1 else CV_S2SQ),
                            BN_EPS, op0=ALU.mult, op1=ALU.add)
    std = mk()
    nc.scalar.activation(std[:], v[:], ACTF.Sqrt, bias=0.0, scale=1.0)
    r = mk()
    nc.vector.reciprocal(r[:], std[:])
    A = mk()
    nc.vector.tensor_tensor(out=A[:], in0=cvcol(CV_G1 if ph == 1 else CV_G2),
                            in1=r[:], op=ALU.mult)
    scale = mk()
    nc.vector.tensor_tensor(out=scale[:], in0=A[:],
                            in1=cvcol(CV_S15 if ph == 1 else CV_S2), op=ALU.mult)
    m1 = mk()
    nc.vector.tensor_tensor(out=m1[:], in0=mean[:], in1=scale[:], op=ALU.mult)
    bias = mk()
    nc.vector.tensor_tensor(out=bias[:], in0=cvcol(CV_B15 if ph == 1 else CV_B2),
                            in1=m1[:], op=ALU.subtract)
    return scale, bias


def build_program(ncores, nper, collective=True, reps=1):
    nc = bacc.Bacc("TRN2", target_bir_lowering=False, debug=False, num_devices=ncores)

    x_in = nc.dram_tensor("x", [nper, C, H, W], F32, kind="ExternalInput")
    w1_in = nc.dram_tensor("w1s", [C, 10, C], F8, kind="ExternalInput")
    w2_in = nc.dram_tensor("w2s", [C, 10, C], F8, kind="ExternalInput")
    cv_in = nc.dram_tensor("cvec", [C, CV_NCOLS], F32, kind="ExternalInput")
    out_d = nc.dram_tensor("out", [nper, C, H, W], BF16, kind="ExternalOutput")

    inv_m = 1.0 / float(ncores * nper * H * W)

    with tile.TileContext(nc) as tc:
        with tc.tile_pool(name="const", bufs=1) as cpool, \
             tc.tile_pool(name="xres", bufs=nper) as xpool, \
             tc.tile_pool(name="cint", bufs=nper) as ipool, \
             tc.tile_pool(name="apad", bufs=nper) as apool, \
             tc.tile_pool(name="u8", bufs=3) as upool, \
             tc.tile_pool(name="sq", bufs=3) as sqpool, \
             tc.tile_pool(name="stat", bufs=1) as spool, \
             tc.tile_pool(name="psum", bufs=1, space="PSUM") as ppool, \
             tc.tile_pool(name="dram", bufs=1, space="DRAM") as dpool:

            tw1 = cpool.tile([C, 10, C], F8, tag="w1")
            tw2 = cpool.tile([C, 10, C], F8, tag="w2")
            tcv = cpool.tile([C, CV_NCOLS], F32, tag="cv")
            nc.sync.dma_start(tw1[:], w1_in.ap())
            nc.sync.dma_start(tw2[:], w2_in.ap())
            nc.sync.dma_start(tcv[:], cv_in.ap())

            def cvcol(j):
                return tcv[:, j:j + 1]

            # pre-warm the Sqrt activation table so BN1 coefs don't pay it
            warm = cpool.tile([C, 1], F32, tag="warm")
            nc.scalar.activation(warm[:], cvcol(CV_S1SQ), ACTF.Sqrt, bias=0.0, scale=1.0)

            xr = [xpool.tile([C, H, W], F32, tag="xr", name=f"xr{i}") for i in range(nper)]
            cint = [ipool.tile([C, H, W], I16, tag="cint", name=f"cint{i}") for i in range(nper)]
            apad = [apool.tile([C, HP, WP], F8, tag="apad", name=f"apad{i}") for i in range(nper)]

            # per-copy sum partials (2 copies per image per conv) + sumsq (1/img)
            p1p = spool.tile([C, 2, 4 * nper], F32, tag="p1p")
            p2p = spool.tile([C, 2, 4 * nper], F32, tag="p1p", name="p2p")
            s1p, ss1p = p1p[:, 0, :], p1p[:, 1, :]
            s2p, ss2p = p2p[:, 0, :], p2p[:, 1, :]

            rep_ctx = tc.For_i(0, reps, 1) if reps > 1 else None
            if rep_ctx is not None:
                rep_ctx.__enter__()

            warm_n = [0]

            def pe_warm(n):
                """Dummy DR matmuls reading the (constant) weight tile: keep the
                tensor engine's p-state ramp alive across idle gaps."""
                warm_n[0] += 1
                pt = ppool.tile([C, 4, GR, WP], F32, tag="pt",
                                name=f"warm{warm_n[0]}", bufs=2)
                flatw = tw1[:].rearrange("c a b -> c (a b)")
                for k in range(n):
                    rhs = flatw[:, 0:W]
                    rhs.ap.insert(1, [WP, GR])
                    rhs.ap.insert(1, [2, 2])
                    nc.tensor.matmul(pt[:, k % 4, :, 0:W], tw1[:, 0:2, :], rhs,
                                     start=True, stop=True, perf_mode=DR)

            def stats_half(i, h, ssp, eng):
                """Half-image sum-of-squares for image i, half h (0/1)."""
                h0 = h * (H // 2)
                view = cint[i][:, h0:h0 + H // 2, :]
                sq = sqpool.tile([C, H // 2, W], BF16, tag="sq")
                k = 4 * i + h
                if eng == "act":
                    nc.scalar.activation(sq[:], view, ACTF.Square, bias=0.0,
                                         scale=1.0, accum_out=ssp[:, k:k + 1])
                else:
                    nc.vector.scalar_tensor_tensor(
                        out=sq[:], in0=view, scalar=1.0, in1=view,
                        op0=ALU.mult, op1=ALU.mult, accum_out=ssp[:, k:k + 1])

            def conv(i, tw, sp):
                """conv3x3 of apad[i] -> cint[i] (int16) + sum partials.
                5 dense DoubleRow matmuls per 8-row group; 2 PSUM tiles of
                4 banks each (groups 0-3 / 4-6)."""
                pts = [ppool.tile([C, 4, GR, WP], F32, tag="pt", name=f"pt{i}_{d}", bufs=2)
                       for d in range(2)]
                flat = apad[i].rearrange("c h w -> c (h w)")
                for g in range(NG):
                    pt = pts[g // 4]
                    out = pt[:, g % 4, :, 0:W]          # [C, 8, 56] in one bank
                    for p, (off, pstride) in enumerate(PAIRS):
                        base = g * GR * WP + off
                        rhs = flat[:, base:base + W]
                        rhs.ap.insert(1, [WP, GR])      # 8 output rows
                        rhs.ap.insert(1, [pstride, 2])  # DR pair
                        nc.tensor.matmul(out, tw[:, 2 * p:2 * p + 2, :], rhs,
                                         start=(p == 0), stop=(p == 4),
                                         perf_mode=DR)
                if i == nper - 1:
                    chunks = [(0, 0, 2), (0, 2, 2), (1, 0, 2), (1, 2, 1)]
                else:
                    chunks = [(0, 0, 4), (1, 0, 3)]
                for d, (ti, g0, rows) in enumerate(chunks):
                    src = pts[ti][:, g0:g0 + rows, :, 0:W]
                    r0 = ti * 32 + g0 * GR
                    dstv = cint[i][:, r0:r0 + rows * GR, :] \
                        .rearrange("c (a b) w -> c a b w", a=rows)
                    k = 4 * i + d
                    if i == nper - 1:
                        # last image: copy on DVE (cuts the Act tail); sums
                        # recovered below from the exact int16 cint
                        nc.vector.tensor_scalar(dstv, src, 1.0, None, op0=ALU.mult)
                    else:
                        nc.scalar.activation(dstv, src, ACTF.Identity, bias=0.0,
                                             scale=1.0, accum_out=sp[:, k:k + 1])
                if i == nper - 1:
                    civ = cint[i][:].rearrange("c h w -> c (h w)")
                    half = H * W // 2
                    k = 4 * i
                    nc.vector.tensor_reduce(out=sp[:, k:k + 1], in_=civ[:, 0:half],
                                            axis=AX.X, op=ALU.add)
                    nc.vector.tensor_reduce(out=sp[:, k + 1:k + 2], in_=civ[:, half:],
                                            axis=AX.X, op=ALU.add)

            def stats_allreduce(pp, tag):
                st = spool.tile([C, 2], F32, tag=f"st{tag}")
                nc.vector.tensor_reduce(out=st[:], in_=pp[:], axis=AX.X, op=ALU.add)
                if not collective:
                    return st
                din = dpool.tile([C, 2], F32, tag=f"din{tag}")
                dout = dpool.tile([C, 2], F32, tag=f"dout{tag}")
                nc.gpsimd.dma_start(din[:], st[:])
                nc.gpsimd.collective_compute(
                    "AllReduce", ALU.add,
                    replica_groups=[list(range(ncores))],
                    ins=[din.opt()], outs=[dout.opt()])
                gst = spool.tile([C, 2], F32, tag=f"gst{tag}")
                nc.gpsimd.dma_start(gst[:], dout[:])
                return gst

            def stats_engines(j, nper):
                # images 0-1 on Act; 2..nper-3 on DVE; last two split Act/DVE
                if j < 2:
                    return ("act", "act")
                if j >= nper - 2:
                    return ("act", "dve")
                return ("dve", "dve")

            # ---------------- stage A (act_quant of x) + conv1 ----------------
            with nc.named_scope("conv1"):
                pe_warm(18)
                HH = H // 2
                for i in range(nper):
                    # zero borders just-in-time, on DVE (keep Pool for min)
                    nc.vector.memset(apad[i][:, 0, :], 0)
                    nc.vector.memset(apad[i][:, HP - 1, :], 0)
                    nc.vector.memset(apad[i][:, 1:57, 0:1], 0)
                    nc.vector.memset(apad[i][:, 1:57, 57:WP], 0)
                    j = i - 2
                    e0 = e1 = None
                    if j >= 0:
                        e0, e1 = stats_engines(j, nper)
                        stats_half(j, 0, ss1p, e0)
                    # finer chunks for image 0 shorten the pipeline prologue
                    CH = 14 if i in (0, nper - 1) else HH
                    for ci, h0 in enumerate(range(0, H, CH)):
                        nc.sync.dma_start(xr[i][:, h0:h0 + CH, :],
                                          x_in.ap()[i][:, h0:h0 + CH, :])
                        u8t = upool.tile([C, HH, W], U8, tag="u8")
                        u8v = u8t[:, 0:CH, :]
                        # u8 = saturate(round(15x)): bottom clip + round
                        nc.vector.tensor_scalar(u8v, xr[i][:, h0:h0 + CH, :],
                                                15.0, None, op0=ALU.mult)
                        # top clip + exact int -> fp8 (Pool; DVE on first/last img)
                        meng = nc.vector if (i in (0, nper - 1) and ci % 2 == 1) else nc.gpsimd
                        meng.tensor_scalar(apad[i][:, 1 + h0:1 + h0 + CH, 1:W + 1],
                                           u8v, 15.0, None, op0=ALU.min)
                    conv(i, tw1, s1p)
                    if j >= 0:
                        stats_half(j, 1, ss1p, e1)
                for j in (nper - 2, nper - 1):
                    e0, e1 = stats_engines(j, nper)
                    stats_half(j, 0, ss1p, e0)
                    stats_half(j, 1, ss1p, e1)

            # ---------------- BN1 sync + coefs ----------------
            with nc.named_scope("bn1"):
                gst1 = stats_allreduce(p1p, 1)
                uscale, ubias = _bn_coefs(nc, spool, gst1[:, 0:1], gst1[:, 1:2],
                                          cvcol, inv_m, 1)
                pe_warm(24)

            # ---------------- phase2 (act_quant of BN1) + conv2 ----------------
            with nc.named_scope("conv2"):
                for i in range(nper):
                    j = i - 2
                    e0 = e1 = None
                    if j >= 0:
                        e0, e1 = stats_engines(j, nper)
                        stats_half(j, 0, ss2p, e0)
                    CH = 14 if i == 0 else HH
                    for ci, h0 in enumerate(range(0, H, CH)):
                        u8t = upool.tile([C, HH, W], U8, tag="u8")
                        u8v = u8t[:, 0:CH, :]
                        nc.vector.tensor_scalar(u8v, cint[i][:, h0:h0 + CH, :],
                                                uscale[:], ubias[:],
                                                op0=ALU.mult, op1=ALU.add)
                        # min: h0 Pool; h1 Pool for img 2-4, DVE for 0-1/5+
                        if ci % 2 == 0 or (2 <= i <= 4):
                            meng = nc.gpsimd
                        else:
                            meng = nc.vector
                        meng.tensor_scalar(apad[i][:, 1 + h0:1 + h0 + CH, 1:W + 1],
                                           u8v, 15.0, None, op0=ALU.min)
                    conv(i, tw2, s2p)
                    if j >= 0:
                        stats_half(j, 1, ss2p, e1)
                for j in (nper - 2, nper - 1):
                    e0, e1 = stats_engines(j, nper)
                    stats_half(j, 0, ss2p, e0)
                    stats_half(j, 1, ss2p, e1)

            # ---------------- BN2 sync + coefs ----------------
            with nc.named_scope("bn2"):
                gst2 = stats_allreduce(p2p, 2)
                fscale, fbias = _bn_coefs(nc, spool, gst2[:, 0:1], gst2[:, 1:2],
                                          cvcol, inv_m, 2)

            # ---------------- finalize: BN2 + residual -> bf16 out ----------------
            with nc.named_scope("finalize"):
                for i in range(nper):
                    t = sqpool.tile([C, H, W], BF16, tag="sq", name=f"fin{i}")
                    if i in (1, 5):
                        # (b): t = cint*fscale + fbias (DVE ts), then t += x (Pool)
                        nc.vector.tensor_scalar(t[:], cint[i][:], fscale[:], fbias[:],
                                                op0=ALU.mult, op1=ALU.add)
                        nc.gpsimd.tensor_tensor(out=t[:], in0=t[:], in1=xr[i][:],
                                                op=ALU.add)
                    else:
                        # (a): xb = x + fbias (Act), then out = cint*fscale + xb (DVE STT)
                        nc.scalar.activation(xr[i][:], xr[i][:], ACTF.Identity,
                                             bias=fbias[:], scale=1.0)
                        nc.vector.scalar_tensor_tensor(
                            out=t[:], in0=cint[i][:], scalar=fscale[:],
                            in1=xr[i][:], op0=ALU.mult, op1=ALU.add)
                    nc.sync.dma_start(out_d.ap()[i], t[:])

            if rep_ctx is not None:
                rep_ctx.__exit__(None, None, None)

    nc.compile()
    return nc


_PROG_CACHE = {}


def _get_program(ncores, nper):
    key = (ncores, nper)
    if key not in _PROG_CACHE:
        _PROG_CACHE[key] = build_program(ncores, nper)
    return _PROG_CACHE[key]


def make_inputs(x, w1, w2, gamma1, beta1, gamma2, beta2, ncores=NCORES):
    """Host-side prep: shard x, quantize weights, build cvec."""
    x = np.asarray(x, dtype=np.float32)
    n = x.shape[0]
    nper = n // ncores
    assert nper * ncores == n

    def wq(w):
        w = np.asarray(w, dtype=np.float32)
        alpha = np.float32(np.abs(w).max()) + np.float32(1e-12)
        q = np.round(np.clip(w / alpha, -1.0, 1.0) * np.float32(7.0))
        return q.astype(np.float32), np.float32(alpha)

    q1, a1 = wq(w1)
    q2, a2 = wq(w2)
    # [co, ci, kh, kw] -> [ci, j, co]: rows 2p,2p+1 = DoubleRow tap pairs
    # [(0,dw),(1,dw)] dw=0..2, [(2,0),(2,2)], [(2,1), zero]
    f8np = mybir.dt.np(F8)
    order = [(0, 0), (1, 0), (0, 1), (1, 1), (0, 2), (1, 2), (2, 0), (2, 2), (2, 1)]

    def pack(q):
        t = q.transpose(1, 2, 3, 0)          # [ci, kh, kw, co]
        rows = [t[:, kh, kw, :] for kh, kw in order]
        rows.append(np.zeros_like(rows[0]))  # zero row pairs with tap (2,1)
        return np.ascontiguousarray(np.stack(rows, axis=1)).astype(f8np)

    w1s = pack(q1)
    w2s = pack(q2)
    s1 = np.float32(a1 / np.float32(105.0))   # alpha/7/15: real = s * conv_int
    s2 = np.float32(a2 / np.float32(105.0))

    cvec = np.zeros((C, CV_NCOLS), dtype=np.float32)
    cvec[:, CV_G1] = np.asarray(gamma1, dtype=np.float32)
    cvec[:, CV_B15] = np.float32(15.0) * np.asarray(beta1, dtype=np.float32)
    cvec[:, CV_G2] = np.asarray(gamma2, dtype=np.float32)
    cvec[:, CV_B2] = np.asarray(beta2, dtype=np.float32)
    cvec[:, CV_S1SQ] = s1 * s1
    cvec[:, CV_S2SQ] = s2 * s2
    cvec[:, CV_S15] = np.float32(15.0) * s1
    cvec[:, CV_S2] = s2

    in_maps = []
    for c in range(ncores):
        in_maps.append({
            "x": np.ascontiguousarray(x[c * nper:(c + 1) * nper]),
            "w1s": w1s, "w2s": w2s, "cvec": cvec,
        })
    return in_maps, nper


def run(x, w1, w2, gamma1, beta1, gamma2, beta2, trace=False):
    in_maps, nper = make_inputs(x, w1, w2, gamma1, beta1, gamma2, beta2)
    nc = _get_program(NCORES, nper)
    res = bass_utils.run_bass_kernel_spmd(
        nc, in_maps, core_ids=list(range(NCORES)), trace=trace)
    out = np.concatenate([np.asarray(r["out"]).astype(np.float32) for r in res.results], axis=0)
    return out, res


def kernel(x, w1, w2, gamma1, beta1, gamma2, beta2):
    out, _ = run(x, w1, w2, gamma1, beta1, gamma2, beta2)
    return out


# revision 40
# speedup vs baseline: 1.0404x; 1.0404x over previous
"""Trainium2 Bass kernel for a quantized ResNet BasicBlock.

Reference computation (per reference.py):
    out = act_quant(x); out = conv3x3(out, weight_quant(w1)); out = BN(out, g1, b1)
    out = act_quant(out); out = conv3x3(out, weight_quant(w2)); out = BN(out, g2, b2)
    return out + x
with act_quant(x) = round(clip(x,0,1)*15)/15 (4-bit), weight_quant symmetric 4-bit
per-tensor (levels -7..7, scale alpha/7, alpha = max|w|), BN in training mode
(batch stats over (N,H,W)).

Strategy (8 NeuronCores, data-parallel over batch, sync-BN via AllReduce):
  * Quantized activations are integers 0..15, weights integers -7..7 - both
    exact in fp8e4m3, and fp32 PSUM accumulation never rounds, so each conv3x3
    is an EXACT integer computation.
  * act_quant in 2 ops: float->uint8 conversion saturates [0,255] and rounds
    RNE (matches jnp.round + bottom clip), then min(u8,15) -> fp8 gives the
    top clip.  No +128 bias trick, no separate clip pass.
  * conv3x3 over a zero-padded [C=128, 58, 64] fp8 image as 5 dense DoubleRow
    pair-matmuls per 8-row group (out [C,8,56], 448 wide): pairs (0,dw)+(1,dw)
    at pair-stride 64, (2,0)+(2,2) at pair-stride 2, and (2,1)+zero-weight-row
    at stride 2.  All 9 taps run at the fp8 DoubleRow rate.
  * PSUM->SBUF copy (Act, accum_out) emits per-channel sums and stores conv
    results as int16; sum-of-squares via DVE scalar_tensor_tensor / Act Square
    (split for engine balance, emitted two images behind the conv so in-order
    engine queues never convoy quant ops behind the heavy stats ops).
    Per-channel sum/sumsq are AllReduced across the 8 cores ([128,2] fp32);
    BN+act_quant collapse into a per-channel scale/bias.
  * Finalize: out = fscale*cint + (x + fbias) via Act bias-add + DVE
    scalar_tensor_tensor (6 images) or DVE affine + Pool add (2 images),
    written as bf16 (host upcasts; ~2e-3 rel err, gate is 2e-2) which halves
    the store DMA.  x stays resident in SBUF the whole time (no reload).
  * Dummy DoubleRow matmuls on the constant weight tile keep the tensor
    engine's p-state ramp alive at program start and across the BN1 barrier.
"""

import os
import sys

for _p in ("/opt/trn_rl_repo", "/root/.axon_site/_ro/trn_rl_repo"):
    if os.path.isdir(_p) and _p not in sys.path:
        sys.path.insert(0, _p)

import numpy as np
import ml_dtypes

import concourse.bass as bass  # noqa: F401  (registers types)
import concourse.tile as tile
from concourse import bacc, mybir
from concourse import bass_utils

F32 = mybir.dt.float32
BF16 = mybir.dt.bfloat16
I16 = mybir.dt.int16
U8 = mybir.dt.uint8
F8 = mybir.dt.float8e4
ACTF = mybir.ActivationFunctionType
ALU = mybir.AluOpType
AX = mybir.AxisListType
DR = mybir.MatmulPerfMode.DoubleRow

C = 128
H = W = 56
HP = 58               # padded rows: 1 top + 56 + 1 bottom
WP = 64               # padded cols (16B-aligned rows)
GR = 8                # output rows per PSUM group
NG = H // GR          # 7 groups per image
NCORES = 8

# cvec column indices (all [C] fp32, host-computed)
CV_G1, CV_B15, CV_G2, CV_B2, CV_S1SQ, CV_S2SQ, CV_S15, CV_S2, CV_NCOLS = range(9)

BN_EPS = 1e-5

# DoubleRow tap pairs: (flat offset within group, pair stride).  Weight rows
# 2p,2p+1 hold the two taps; row 9 is all-zero (pairs tap (2,1) with garbage).
PAIRS = [(0 * WP + 0, WP),   # (0,0)+(1,0)
         (0 * WP + 1, WP),   # (0,1)+(1,1)
         (0 * WP + 2, WP),   # (0,2)+(1,2)
         (2 * WP + 0, 2),    # (2,0)+(2,2)
         (2 * WP + 1, 2)]    # (2,1)+zero


def _bn_coefs(nc, pool, S, SS, cvcol, inv_m, ph):
    """[C,1] coef math from global integer-unit sum S / sumsq SS.

    ph=1: (uscale, ubias) with u = conv_int*uscale + ubias = 15*BN(y); u8
          conversion then rounds and bottom-clips, min(,15) top-clips.
    ph=2: (fscale, fbias) with out = conv_int*fscale + fbias = BN(y2).
    """
    idx = [0]

    def mk():
        idx[0] += 1
        return pool.tile([C, 1], F32, tag=f"bc{ph}_{idx[0]}", name=f"bc{ph}_{idx[0]}")

    mean = mk()
    nc.vector.tensor_scalar(mean[:], S, inv_m, None, op0=ALU.mult)
    e2 = mk()
    nc.vector.tensor_scalar(e2[:], SS, inv_m, None, op0=ALU.mult)
    msq = mk()
    nc.vector.tensor_tensor(out=msq[:], in0=mean[:], in1=mean[:], op=ALU.mult)
    var = mk()
    nc.vector.tensor_tensor(out=var[:], in0=e2[:], in1=msq[:], op=ALU.subtract)
    v = mk()
    nc.vector.tensor_scalar(v[:], var[:], cvcol(CV_S1SQ if ph == 1 else CV_S2SQ),
                            BN_EPS, op0=ALU.mult, op1=ALU.add)
    std = mk()
    nc.scalar.activation(std[:], v[:], ACTF.Sqrt, bias=0.0, scale=1.0)
    r = mk()
    nc.vector.reciprocal(r[:], std[:])
    A = mk()
    nc.vector.tensor_tensor(out=A[:], in0=cvcol(CV_G1 if ph == 1 else CV_G2),
                            in1=r[:], op=ALU.mult)
    scale = mk()
    nc.vector.tensor_tensor(out=scale[:], in0=A[:],
                            in1=cvcol(CV_S15 if ph == 1 else CV_S2), op=ALU.mult)
    m1 = mk()
    nc.vector.tensor_tensor(out=m1[:], in0=mean[:], in1=scale[:], op=ALU.mult)
    bias = mk()
    nc.vector.tensor_tensor(out=bias[:], in0=cvcol(CV_B15 if ph == 1 else CV_B2),
                            in1=m1[:], op=ALU.subtract)
    return scale, bias


def build_program(ncores, nper, collective=True, reps=1):
    nc = bacc.Bacc("TRN2", target_bir_lowering=False, debug=False, num_devices=ncores)

    x_in = nc.dram_tensor("x", [nper, C, H, W], F32, kind="ExternalInput")
    w1_in = nc.dram_tensor("w1s", [C, 10, C], F8, kind="ExternalInput")
    w2_in = nc.dram_tensor("w2s", [C, 10, C], F8, kind="ExternalInput")
    cv_in = nc.dram_tensor("cvec", [C, CV_NCOLS], F32, kind="ExternalInput")
    out_d = nc.dram_tensor("out", [nper, C, H, W], BF16, kind="ExternalOutput")

    inv_m = 1.0 / float(ncores * nper * H * W)

    with tile.TileContext(nc) as tc:
        with tc.tile_pool(name="const", bufs=1) as cpool, \
             tc.tile_pool(name="xres", bufs=nper) as xpool, \
             tc.tile_pool(name="cint", bufs=nper) as ipool, \
             tc.tile_pool(name="apad", bufs=nper) as apool, \
             tc.tile_pool(name="u8", bufs=3) as upool, \
             tc.tile_pool(name="sq", bufs=3) as sqpool, \
             tc.tile_pool(name="stat", bufs=1) as spool, \
             tc.tile_pool(name="psum", bufs=1, space="PSUM") as ppool, \
             tc.tile_pool(name="dram", bufs=1, space="DRAM") as dpool:

            tw1 = cpool.tile([C, 10, C], F8, tag="w1")
            tw2 = cpool.tile([C, 10, C], F8, tag="w2")
            tcv = cpool.tile([C, CV_NCOLS], F32, tag="cv")
            nc.sync.dma_start(tw1[:], w1_in.ap())
            nc.sync.dma_start(tw2[:], w2_in.ap())
            nc.sync.dma_start(tcv[:], cv_in.ap())

            def cvcol(j):
                return tcv[:, j:j + 1]

            # pre-warm the Sqrt activation table so BN1 coefs don't pay it
            warm = cpool.tile([C, 1], F32, tag="warm")
            nc.scalar.activation(warm[:], cvcol(CV_S1SQ), ACTF.Sqrt, bias=0.0, scale=1.0)

            xr = [xpool.tile([C, H, W], F32, tag="xr", name=f"xr{i}") for i in range(nper)]
            cint = [ipool.tile([C, H, W], I16, tag="cint", name=f"cint{i}") for i in range(nper)]
            apad = [apool.tile([C, HP, WP], F8, tag="apad", name=f"apad{i}") for i in range(nper)]

            # per-copy sum partials (2 copies per image per conv) + sumsq (1/img)
            p1p = spool.tile([C, 2, 4 * nper], F32, tag="p1p")
            p2p = spool.tile([C, 2, 4 * nper], F32, tag="p1p", name="p2p")
            s1p, ss1p = p1p[:, 0, :], p1p[:, 1, :]
            s2p, ss2p = p2p[:, 0, :], p2p[:, 1, :]

            rep_ctx = tc.For_i(0, reps, 1) if reps > 1 else None
            if rep_ctx is not None:
                rep_ctx.__enter__()

            warm_n = [0]

            def pe_warm(n):
                """Dummy DR matmuls reading the (constant) weight tile: keep the
                tensor engine's p-state ramp alive across idle gaps."""
                warm_n[0] += 1
                pt = ppool.tile([C, 4, GR, WP], F32, tag="pt",
                                name=f"warm{warm_n[0]}", bufs=2)
                flatw = tw1[:].rearrange("c a b -> c (a b)")
                for k in range(n):
                    rhs = flatw[:, 0:W]
                    rhs.ap.insert(1, [WP, GR])
                    rhs.ap.insert(1, [2, 2])
                    nc.tensor.matmul(pt[:, k % 4, :, 0:W], tw1[:, 0:2, :], rhs,
                                     start=True, stop=True, perf_mode=DR)

            def stats_half(i, h, ssp, eng):
                """Half-image sum-of-squares for image i, half h (0/1)."""
                h0 = h * (H // 2)
                view = cint[i][:, h0:h0 + H // 2, :]
                sq = sqpool.tile([C, H // 2, W], BF16, tag="sq")
                k = 4 * i + h
                if eng == "act":
                    nc.scalar.activation(sq[:], view, ACTF.Square, bias=0.0,
                                         scale=1.0, accum_out=ssp[:, k:k + 1])
                else:
                    nc.vector.scalar_tensor_tensor(
                        out=sq[:], in0=view, scalar=1.0, in1=view,
                        op0=ALU.mult, op1=ALU.mult, accum_out=ssp[:, k:k + 1])

            def conv(i, tw, sp):
                """conv3x3 of apad[i] -> cint[i] (int16) + sum partials.
                5 dense DoubleRow matmuls per 8-row group; 2 PSUM tiles of
                4 banks each (groups 0-3 / 4-6)."""
                pts = [ppool.tile([C, 4, GR, WP], F32, tag="pt", name=f"pt{i}_{d}", bufs=2)
                       for d in range(2)]
                flat = apad[i].rearrange("c h w -> c (h w)")
                for g in range(NG):
                    pt = pts[g // 4]
                    out = pt[:, g % 4, :, 0:W]          # [C, 8, 56] in one bank
                    for p, (off, pstride) in enumerate(PAIRS):
                        base = g * GR * WP + off
                        rhs = flat[:, base:base + W]
                        rhs.ap.insert(1, [WP, GR])      # 8 output rows
                        rhs.ap.insert(1, [pstride, 2])  # DR pair
                        nc.tensor.matmul(out, tw[:, 2 * p:2 * p + 2, :], rhs,
                                         start=(p == 0), stop=(p == 4),
                                         perf_mode=DR)
                if i == nper - 1:
                    chunks = [(0, 0, 2), (0, 2, 2), (1, 0, 2), (1, 2, 1)]
                else:
                    chunks = [(0, 0, 4), (1, 0, 3)]
                for d, (ti, g0, rows) in enumerate(chunks):
                    src = pts[ti][:, g0:g0 + rows, :, 0:W]
                    r0 = ti * 32 + g0 * GR
                    dstv = cint[i][:, r0:r0 + rows * GR, :] \
                        .rearrange("c (a b) w -> c a b w", a=rows)
                    k = 4 * i + d
                    nc.scalar.activation(dstv, src, ACTF.Identity, bias=0.0,
                                         scale=1.0, accum_out=sp[:, k:k + 1])

            def stats_allreduce(pp, tag):
                st = spool.tile([C, 2], F32, tag=f"st{tag}")
                nc.vector.tensor_reduce(out=st[:], in_=pp[:], axis=AX.X, op=ALU.add)
                if not collective:
                    return st
                din = dpool.tile([C, 2], F32, tag=f"din{tag}")
                dout = dpool.tile([C, 2], F32, tag=f"dout{tag}")
                nc.gpsimd.dma_start(din[:], st[:])
                nc.gpsimd.collective_compute(
                    "AllReduce", ALU.add,
                    replica_groups=[list(range(ncores))],
                    ins=[din.opt()], outs=[dout.opt()])
                gst = spool.tile([C, 2], F32, tag=f"gst{tag}")
                nc.gpsimd.dma_start(gst[:], dout[:])
                return gst

            def stats_engines(j, nper):
                # images 0-1 on Act; 2..nper-3 on DVE; last two split Act/DVE
                if j < 2:
                    return ("act", "act")
                if j >= nper - 2:
                    return ("act", "dve")
                return ("dve", "dve")

            # ---------------- stage A (act_quant of x) + conv1 ----------------
            with nc.named_scope("conv1"):
                pe_warm(18)
                HH = H // 2
                for i in range(nper):
                    # zero borders just-in-time, on DVE (keep Pool for min)
                    nc.vector.memset(apad[i][:, 0, :], 0)
                    nc.vector.memset(apad[i][:, HP - 1, :], 0)
                    nc.vector.memset(apad[i][:, 1:57, 0:1], 0)
                    nc.vector.memset(apad[i][:, 1:57, 57:WP], 0)
                    j = i - 2
                    e0 = e1 = None
                    if j >= 0:
                        e0, e1 = stats_engines(j, nper)
                        stats_half(j, 0, ss1p, e0)
                    # finer chunks for image 0 shorten the pipeline prologue
                    CH = 14 if i in (0, nper - 1) else HH
                    for ci, h0 in enumerate(range(0, H, CH)):
                        nc.sync.dma_start(xr[i][:, h0:h0 + CH, :],
                                          x_in.ap()[i][:, h0:h0 + CH, :])
                        u8t = upool.tile([C, HH, W], U8, tag="u8")
                        u8v = u8t[:, 0:CH, :]
                        # u8 = saturate(round(15x)): bottom clip + round
                        nc.vector.tensor_scalar(u8v, xr[i][:, h0:h0 + CH, :],
                                                15.0, None, op0=ALU.mult)
                        # top clip + exact int -> fp8 (Pool; DVE on first/last img)
                        meng = nc.vector if (i in (0, nper - 1) and ci % 2 == 1) else nc.gpsimd
                        meng.tensor_scalar(apad[i][:, 1 + h0:1 + h0 + CH, 1:W + 1],
                                           u8v, 15.0, None, op0=ALU.min)
                    conv(i, tw1, s1p)
                    if j >= 0:
                        stats_half(j, 1, ss1p, e1)
                for j in (nper - 2, nper - 1):
                    e0, e1 = stats_engines(j, nper)
                    stats_half(j, 0, ss1p, e0)
                    stats_half(j, 1, ss1p, e1)

            # ---------------- BN1 sync + coefs ----------------
            with nc.named_scope("bn1"):
                gst1 = stats_allreduce(p1p, 1)
                uscale, ubias = _bn_coefs(nc, spool, gst1[:, 0:1], gst1[:, 1:2],
                                          cvcol, inv_m, 1)
                pe_warm(40)

            # ---------------- phase2 (act_quant of BN1) + conv2 ----------------
            with nc.named_scope("conv2"):
                for i in range(nper):
                    j = i - 2
                    e0 = e1 = None
                    if j >= 0:
                        e0, e1 = stats_engines(j, nper)
                        stats_half(j, 0, ss2p, e0)
                    CH = 14 if i == 0 else HH
                    for ci, h0 in enumerate(range(0, H, CH)):
                        u8t = upool.tile([C, HH, W], U8, tag="u8")
                        u8v = u8t[:, 0:CH, :]
                        nc.vector.tensor_scalar(u8v, cint[i][:, h0:h0 + CH, :],
                                                uscale[:], ubias[:],
                                                op0=ALU.mult, op1=ALU.add)
                        # min: h0 Pool; h1 Pool for img 2-4, DVE for 0-1/5+
                        if ci % 2 == 0 or (2 <= i <= 4):
                            meng = nc.gpsimd
                        else:
                            meng = nc.vector
                        meng.tensor_scalar(apad[i][:, 1 + h0:1 + h0 + CH, 1:W + 1],
                                           u8v, 15.0, None, op0=ALU.min)
                    conv(i, tw2, s2p)
                    if j >= 0:
                        stats_half(j, 1, ss2p, e1)
                for j in (nper - 2, nper - 1):
                    e0, e1 = stats_engines(j, nper)
                    stats_half(j, 0, ss2p, e0)
                    stats_half(j, 1, ss2p, e1)

            # ---------------- BN2 sync + coefs ----------------
            with nc.named_scope("bn2"):
                gst2 = stats_allreduce(p2p, 2)
                fscale, fbias = _bn_coefs(nc, spool, gst2[:, 0:1], gst2[:, 1:2],
                                          cvcol, inv_m, 2)

            # ---------------- finalize: BN2 + residual -> bf16 out ----------------
            with nc.named_scope("finalize"):
                for i in range(nper):
                    t = sqpool.tile([C, H, W], BF16, tag="sq", name=f"fin{i}")
                    if i in (1, 5):
                        # (b): t = cint*fscale + fbias (DVE ts), then t += x (Pool)
                        nc.vector.tensor_scalar(t[:], cint[i][:], fscale[:], fbias[:],
                                                op0=ALU.mult, op1=ALU.add)
                        nc.gpsimd.tensor_tensor(out=t[:], in0=t[:], in1=xr[i][:],
                                                op=ALU.add)
                    else:
                        # (a): xb = x + fbias (Act), then out = cint*fscale + xb (DVE STT)
                        nc.scalar.activation(xr[i][:], xr[i][:], ACTF.Identity,
                                             bias=fbias[:], scale=1.0)
                        nc.vector.scalar_tensor_tensor(
                            out=t[:], in0=cint[i][:], scalar=fscale[:],
                            in1=xr[i][:], op0=ALU.mult, op1=ALU.add)
                    nc.sync.dma_start(out_d.ap()[i], t[:])

            if rep_ctx is not None:
                rep_ctx.__exit__(None, None, None)

    nc.compile()
    return nc


_PROG_CACHE = {}


def _get_program(ncores, nper):
    key = (ncores, nper)
    if key not in _PROG_CACHE:
        _PROG_CACHE[key] = build_program(ncores, nper)
    return _PROG_CACHE[key]


def make_inputs(x, w1, w2, gamma1, beta1, gamma2, beta2, ncores=NCORES):
    """Host-side prep: shard x, quantize weights, build cvec."""
    x = np.asarray(x, dtype=np.float32)
    n = x.shape[0]
    nper = n // ncores
    assert nper * ncores == n

    def wq(w):
        w = np.asarray(w, dtype=np.float32)
        alpha = np.float32(np.abs(w).max()) + np.float32(1e-12)
        q = np.round(np.clip(w / alpha, -1.0, 1.0) * np.float32(7.0))
        return q.astype(np.float32), np.float32(alpha)

    q1, a1 = wq(w1)
    q2, a2 = wq(w2)
    # [co, ci, kh, kw] -> [ci, j, co]: rows 2p,2p+1 = DoubleRow tap pairs
    # [(0,dw),(1,dw)] dw=0..2, [(2,0),(2,2)], [(2,1), zero]
    f8np = mybir.dt.np(F8)
    order = [(0, 0), (1, 0), (0, 1), (1, 1), (0, 2), (1, 2), (2, 0), (2, 2), (2, 1)]

    def pack(q):
        t = q.transpose(1, 2, 3, 0)          # [ci, kh, kw, co]
        rows = [t[:, kh, kw, :] for kh, kw in order]
        rows.append(np.zeros_like(rows[0]))  # zero row pairs with tap (2,1)
        return np.ascontiguousarray(np.stack(rows, axis=1)).astype(f8np)

    w1s = pack(q1)
    w2s = pack(q2)
    s1 = np.float32(a1 / np.float32(105.0))   # alpha/7/15: real = s * conv_int
    s2 = np.float32(a2 / np.float32(105.0))

    cvec = np.zeros((C, CV_NCOLS), dtype=np.float32)
    cvec[:, CV_G1] = np.asarray(gamma1, dtype=np.float32)
    cvec[:, CV_B15] = np.float32(15.0) * np.asarray(beta1, dtype=np.float32)
    cvec[:, CV_G2] = np.asarray(gamma2, dtype=np.float32)
    cvec[:, CV_B2] = np.asarray(beta2, dtype=np.float32)
    cvec[:, CV_S1SQ] = s1 * s1
    cvec[:, CV_S2SQ] = s2 * s2
    cvec[:, CV_S15] = np.float32(15.0) * s1
    cvec[:, CV_S2] = s2

    in_maps = []
    for c in range(ncores):
        in_maps.append({
            "x": np.ascontiguousarray(x[c * nper:(c + 1) * nper]),
            "w1s": w1s, "w2s": w2s, "cvec": cvec,
        })
    return in_maps, nper


def run(x, w1, w2, gamma1, beta1, gamma2, beta2, trace=False):
    in_maps, nper = make_inputs(x, w1, w2, gamma1, beta1, gamma2, beta2)
    nc = _get_program(NCORES, nper)
    res = bass_utils.run_bass_kernel_spmd(
        nc, in_maps, core_ids=list(range(NCORES)), trace=trace)
    out = np.concatenate([np.asarray(r["out"]).astype(np.float32) for r in res.results], axis=0)
    return out, res


def kernel(x, w1, w2, gamma1, beta1, gamma2, beta2):
    out, _ = run(x, w1, w2, gamma1, beta1, gamma2, beta2)
    return out
